# revision 4
# baseline (speedup 1.0000x reference)
"""Trainium2 Bass kernel for chunked linear attention (nn_LinearAttention).

Sharding: 8 cores = 2 batches x 4 head-groups (4 heads / 256 feature cols each).
Each core computes q/k/v projections for its head slice, the per-head causal
linear-attention scan (chunked: intra-chunk causal matmul + running KV state),
and a full-width partial of the output projection. Host sums the 4 partials
per batch (the reduction of the head-parallel decomposition).
"""

import sys

sys.path.insert(0, "/opt/trn_rl_repo")
import numpy as np

P = 128
S = 2048
DM = 1024
CW = 256  # feature cols per core (4 heads x 64)
NCH = 16  # chunks of 128 positions
EPS = 1e-6

_cache = {}


def _build():
    if "nc" in _cache:
        return _cache["nc"]
    import concourse.mybir as mybir
    import concourse.tile as tile
    from concourse import bacc
    from concourse.masks import make_identity, make_upper_triangular

    f32 = mybir.dt.float32
    f32r = mybir.dt.float32r
    AO = mybir.AluOpType

    nc = bacc.Bacc("TRN2", target_bir_lowering=False, debug=False)
    xT = nc.dram_tensor("xT", [DM, S], f32r, kind="ExternalInput").ap()
    wqT = nc.dram_tensor("wqT", [DM, CW], f32r, kind="ExternalInput").ap()
    wkT = nc.dram_tensor("wkT", [DM, CW], f32r, kind="ExternalInput").ap()
    wvT = nc.dram_tensor("wvT", [DM, CW], f32r, kind="ExternalInput").ap()
    woT = nc.dram_tensor("woT", [CW, DM], f32r, kind="ExternalInput").ap()
    outp = nc.dram_tensor("outp", [S, DM], f32, kind="ExternalOutput").ap()

    with tile.TileContext(nc) as tc:
        with (
            tc.tile_pool(name="const", bufs=1) as const,
            tc.tile_pool(name="xw", bufs=1) as xw,
            tc.tile_pool(name="qk", bufs=1) as qk,
            tc.tile_pool(name="spool", bufs=4) as spool,
            tc.tile_pool(name="bpool", bufs=4) as bpool,
            tc.tile_pool(name="kpool", bufs=3) as kpool,
            tc.tile_pool(name="onp", bufs=3) as onp,
            tc.tile_pool(name="otp", bufs=3) as otp,
            tc.tile_pool(name="obp", bufs=3) as obp,
            tc.tile_pool(name="rp", bufs=4) as rp,
            tc.tile_pool(name="pp", bufs=3, space="PSUM") as pp,
            tc.tile_pool(name="sm", bufs=3, space="PSUM") as sm,
            tc.tile_pool(name="op", bufs=2, space="PSUM") as op,
        ):
            mask = const.tile([P, P], f32)
            make_upper_triangular(nc, mask[:], val=1.0, diag=True)
            ident = const.tile([P, P], f32)
            make_identity(nc, ident[:])
            ones_f32 = const.tile([P, 128], f32)
            nc.vector.memset(ones_f32[:], 1.0)
            zeros_f32 = const.tile([P, 66], f32)
            nc.vector.memset(zeros_f32[:], 0.0)

            xsb = xw.tile([P, 8, S], f32r)
            nc.sync.dma_start(out=xsb[:], in_=xT.rearrange("(t p) s -> p t s", p=P))
            wq_sb = xw.tile([P, 8, CW], f32r)
            nc.sync.dma_start(out=wq_sb[:], in_=wqT.rearrange("(t p) f -> p t f", p=P))
            wk_sb = xw.tile([P, 8, CW], f32r)
            nc.sync.dma_start(out=wk_sb[:], in_=wkT.rearrange("(t p) f -> p t f", p=P))
            wv_sb = xw.tile([P, 8, CW], f32r)
            nc.sync.dma_start(out=wv_sb[:], in_=wvT.rearrange("(t p) f -> p t f", p=P))
            wo_sb = xw.tile([P, 2, DM], f32r)
            nc.sync.dma_start(out=wo_sb[:], in_=woT.rearrange("(t p) f -> p t f", p=P))

            QT = qk.tile([P, 2, S], f32r)
            KT = qk.tile([P, 2, S], f32r)
            V = qk.tile([P, NCH, 4, 66], f32r)
            nc.vector.tensor_copy(
                V[:, :, :, 64:66],
                ones_f32[:].rearrange("p (a b c) -> p a b c", a=NCH, b=4),
            )

            # q/k projections -> feature-major QT/KT with relu(x)+eps
            for w_sb, dst in ((wq_sb, QT), (wk_sb, KT)):
                for ft in range(2):
                    for pt in range(4):
                        ps = pp.tile([P, 512], f32, tag="pp")
                        for kt in range(8):
                            nc.tensor.matmul(
                                ps[:],
                                w_sb[:, kt, ft * P : (ft + 1) * P],
                                xsb[:, kt, pt * 512 : (pt + 1) * 512],
                                start=(kt == 0),
                                stop=(kt == 7),
                            )
                        nc.vector.tensor_scalar(
                            dst[:, ft, pt * 512 : (pt + 1) * 512],
                            ps[:],
                            0.0,
                            EPS,
                            op0=AO.max,
                            op1=AO.add,
                        )
            # v projection -> position-major V (plus ones column per head)
            for pt in range(NCH):
                ps = pp.tile([P, 256], f32, tag="pp")
                for kt in range(8):
                    nc.tensor.matmul(
                        ps[:],
                        xsb[:, kt, pt * P : (pt + 1) * P],
                        wv_sb[:, kt, :],
                        start=(kt == 0),
                        stop=(kt == 7),
                    )
                nc.scalar.copy(
                    V[:, pt, :, 0:64], ps[:].rearrange("p (h m) -> p h m", h=4)
                )

            # attention scan over chunks; S state per head-pair in SBUF
            s_prev = []
            for p in range(2):
                st = spool.tile([P, 66], f32r, tag="s")
                nc.vector.tensor_copy(st[:], zeros_f32[:])
                s_prev.append(st)

            for c in range(NCH):
                out_ps = op.tile([P, 264], f32, tag="o")
                for p in range(2):
                    ktp = sm.tile([P, P], f32, tag="sm")
                    nc.tensor.transpose(
                        ktp[:], KT[:, p, c * P : (c + 1) * P].bitcast(f32), ident[:]
                    )
                    kpos = kpool.tile([P, P], f32, tag="kp")
                    nc.scalar.copy(kpos[:], ktp[:])
                    for hl in range(2):
                        h = 2 * p + hl
                        qh = QT[hl * 64 : (hl + 1) * 64, p, c * P : (c + 1) * P]
                        kh = KT[hl * 64 : (hl + 1) * 64, p, c * P : (c + 1) * P]
                        atp = sm.tile([P, P], f32, tag="sm")
                        nc.tensor.matmul(
                            atp[:], kh, qh,
                            start=True, stop=True,
                        )
                        bt = bpool.tile([P, P], f32r, tag="bt")
                        nc.vector.tensor_mul(bt[:], atp[:], mask[:])
                        osl = out_ps[:, h * 66 : (h + 1) * 66]
                        nc.tensor.matmul(
                            osl, bt[:], V[:, c, h, :],
                            start=True, stop=False,
                        )
                        nc.tensor.matmul(
                            osl,
                            qh,
                            s_prev[p][hl * 64 : (hl + 1) * 64, :],
                            start=False,
                            stop=True,
                            skip_group_check=True,
                        )
                    ktvp = sm.tile([P, 66], f32, tag="sm")
                    for hl in range(2):
                        h = 2 * p + hl
                        nc.tensor.matmul(
                            ktvp[hl * 64 : (hl + 1) * 64, :],
                            kpos[:, hl * 64 : (hl + 1) * 64],
                            V[:, c, h, :].bitcast(f32),
                            start=True,
                            stop=True,
                        )
                    if c < NCH - 1:
                        s_new = spool.tile([P, 66], f32r, tag="s")
                        nc.vector.tensor_add(s_new[:], s_prev[p][:], ktvp[:])
                        s_prev[p] = s_new

                opsr = out_ps[:].rearrange("p (h m) -> p h m", h=4)
                zt = rp.tile([P, 4], f32, tag="z")
                nc.vector.tensor_scalar_add(zt[:], opsr[:, :, 64], EPS)
                rt = rp.tile([P, 4], f32, tag="r")
                nc.vector.reciprocal(rt[:], zt[:])
                onorm = onp.tile([P, 4, 64], f32, tag="on")
                for h in range(4):
                    nc.vector.tensor_scalar_mul(
                        onorm[:, h, :], opsr[:, h, 0:64], rt[:, h : h + 1]
                    )
                ott = otp.tile([P, 2, P], f32r, tag="ot")
                onf = onorm[:].rearrange("p h m -> p (h m)")
                for f in range(2):
                    otps = sm.tile([P, P], f32, tag="sm")
                    nc.tensor.transpose(
                        otps[:], onf[:, f * P : (f + 1) * P], ident[:]
                    )
                    nc.scalar.copy(ott[:, f, :], otps[:])
                ob = obp.tile([P, DM], f32, tag="ob")
                for n in range(2):
                    prp = op.tile([P, 512], f32, tag="o")
                    for f in range(2):
                        nc.tensor.matmul(
                            prp[:],
                            ott[:, f, :],
                            wo_sb[:, f, n * 512 : (n + 1) * 512],
                            start=(f == 0),
                            stop=(f == 1),
                        )
                    nc.scalar.copy(ob[:, n * 512 : (n + 1) * 512], prp[:])
                nc.sync.dma_start(out=outp[c * P : (c + 1) * P, :], in_=ob[:])

    nc.compile()
    _cache["nc"] = nc
    return nc


def kernel(x, Wq, Wk, Wv, Wo):
    from concourse.bass_utils import run_bass_kernel_spmd

    nc = _build()
    x = np.asarray(x, dtype=np.float32)
    in_maps = []
    for c in range(8):
        b, hg = c // 4, c % 4
        cols = slice(hg * CW, (hg + 1) * CW)
        in_maps.append(
            {
                "xT": np.ascontiguousarray(np.asarray(x[b]).T),
                "wqT": np.ascontiguousarray(np.asarray(Wq)[cols, :].T),
                "wkT": np.ascontiguousarray(np.asarray(Wk)[cols, :].T),
                "wvT": np.ascontiguousarray(np.asarray(Wv)[cols, :].T),
                "woT": np.ascontiguousarray(np.asarray(Wo)[:, cols].T),
            }
        )
    res = run_bass_kernel_spmd(nc, in_maps, core_ids=list(range(8)))
    out = np.zeros((2, S, DM), np.float32)
    for c in range(8):
        out[c // 4] += res.results[c]["outp"]
    return out
